# revision 22
# baseline (speedup 1.0000x reference)
"""Mixtral MoE layer (T=1024, H=1024, I=2048, E=8, top-2) on 8 Trainium2 cores.

Strategy: token-sparse expert-parallel. The router (softmax + top-2 +
renormalize -> combine[T, E]) runs on host. Core c owns expert c's FFN and
processes only the tokens routed to expert c (on average T*K/E = 256,
padded to a fixed bucket NCAP=384; zero-padded columns contribute nothing).
Host gathers each expert's token columns of x^T (the "token all-to-all"
shard step), the device computes

    outT_c = (w2_c @ (silu(w1_c @ xg) * (w3_c @ xg))) * combine[toks_c, c]

and host scatter-adds the per-expert [H, n_c] panels back into the full
[T, H] output (the unshard step). If any expert overflows the bucket
(never for 8 experts at these sizes unless routing is degenerate), we fall
back to a dense variant: every core processes all T tokens with its
combine column, same scatter-add (toks = arange(T)).

Matmuls run as float32r (TF32-like precision, ~2.5e-4 rel err end to end,
full PE rate for moving dims >= 256). Weights are repacked on host so each
i-tile's w1/w3 lhsT blocks and w2 rows form one contiguous [128, 12KB]
DMA (~1.5 MiB per dma_start, descriptor-efficient).
"""

import os
import sys

sys.path.insert(0, "/opt/trn_rl_repo")

import numpy as np

import concourse.bacc as bacc
import concourse.tile as tile
from concourse import mybir
from concourse.bass_utils import run_bass_kernel_spmd

F32 = mybir.dt.float32
F32R = mybir.dt.float32r

T = 1024   # tokens
H = 1024   # hidden
I = 2048   # intermediate
E = 8      # experts
TOPK = 2
P = 128
NKH = H // P     # 8  h-tiles (up-proj contraction)
NI = I // P      # 16 i-tiles
NH = H // P      # 8  h-tiles (down-proj output)
N_CORES = 8
NCAP = 384       # token bucket per expert (seed-robust: mean 256, std ~14)
WCOLS = 3 * NKH * P  # packed weight row: w1 blocks | w3 blocks | w2 rows

_NC_CACHE = {}


def build_nc(ncap: int):
    if ncap in _NC_CACHE:
        return _NC_CACHE[ncap]

    # moving-operand blocks of <=512 (>=256 keeps float32r at full rate)
    nblk = (ncap + 511) // 512
    blks = []
    for b in range(nblk):
        lo = b * 512
        blks.append(slice(lo, min(lo + 512, ncap)))

    # SBUF gets tight in the dense fallback (ncap=T); shrink stream depths
    wq_bufs = 8 if ncap <= 512 else 2
    sm_bufs = 2 if ncap <= 512 else 1

    nc = bacc.Bacc(None, target_bir_lowering=False, num_devices=N_CORES)

    xg_in = nc.declare_dram_parameter("xg", [H, ncap], F32, isOutput=False)
    comb_in = nc.declare_dram_parameter("comb", [P, ncap], F32, isOutput=False)
    # per i-tile packed up-proj weights: [w1 lhsT (NKH*P) | w3 lhsT (NKH*P)]
    w13_in = nc.declare_dram_parameter("w13q", [NI, P, 2 * NKH * P], F32, isOutput=False)
    # down-proj rows in 4 coalesced groups of 4 i-tiles, streamed after the
    # w13 stream (overlaps stage 2)
    w2_in = nc.declare_dram_parameter("w2q", [NI // 4, P, 4 * H], F32, isOutput=False)
    outp = nc.declare_dram_parameter("outp", [H, ncap], F32, isOutput=True)

    with tile.TileContext(nc) as tc:
        with (
            tc.tile_pool(name="persist", bufs=1) as persist,
            tc.tile_pool(name="stream", bufs=2) as stream,
            tc.tile_pool(name="psum", bufs=1, space="PSUM") as psum,
        ):
            # first i-tile's weights ahead of the xg block so PE starts early;
            # split so the w1 blocks (first consumers) land first
            WB = NKH * P
            wq0 = stream.tile([P, 2 * WB], F32R, name="wq_0", tag="wq", bufs=wq_bufs)
            nc.sync.dma_start(out=wq0[:, 0:WB], in_=w13_in[0][:, 0:WB].bitcast(F32R))

            xg_sb = []

            def load_xg(kh):
                t_ = persist.tile([P, ncap], F32R, name=f"xg_{kh}", tag=f"xg_{kh}")
                nc.sync.dma_start(
                    out=t_[:], in_=xg_in[kh * P : (kh + 1) * P, :].bitcast(F32R)
                )
                xg_sb.append(t_)

            for kh in range(3):
                load_xg(kh)
            nc.sync.dma_start(
                out=wq0[:, WB : 2 * WB], in_=w13_in[0][:, WB : 2 * WB].bitcast(F32R)
            )
            load_xg(3)
            wq1 = stream.tile([P, 2 * WB], F32R, name="wq_1", tag="wq", bufs=wq_bufs)
            nc.sync.dma_start(out=wq1[:], in_=w13_in[1].bitcast(F32R))
            for kh in range(4, NKH):
                load_xg(kh)
            comb_sb = persist.tile([P, ncap], F32, name="comb_sb", tag="comb_sb")
            nc.sync.dma_start(out=comb_sb[:], in_=comb_in[:, :])

            w2_sb = []
            act_sb = []

            # ---- stage 1: up-projections + SwiGLU, per i-tile ----
            for it in range(NI):
                if it == 0:
                    wq = wq0
                elif it == 1:
                    wq = wq1
                else:
                    wq = stream.tile([P, 2 * WB], F32R, name=f"wq_{it}", tag="wq", bufs=wq_bufs)
                    nc.sync.dma_start(out=wq[:], in_=w13_in[it].bitcast(F32R))

                # 4 psum tags cycle between ph1/ph3 (alternating => double
                # buffered) and are reused by stage 2's 4 accumulators
                ph1 = psum.tile([P, ncap], F32, name=f"ph1_{it}",
                                tag=f"ps{'AB'[it % 2]}", bufs=1)
                ph3 = psum.tile([P, ncap], F32, name=f"ph3_{it}",
                                tag=f"ps{'CD'[it % 2]}", bufs=1)
                # alternate the two accumulator banks so consecutive PE
                # writes never target the same PSUM bank
                for kh in range(NKH):
                    for ph, woff in ((ph1, 0), (ph3, NKH * P)):
                        lhsT = wq[:, woff + kh * P : woff + (kh + 1) * P]
                        for ts in blks:
                            nc.tensor.matmul(
                                ph[:, ts], lhsT, xg_sb[kh][:, ts],
                                start=(kh == 0), stop=(kh == NKH - 1),
                            )

                silu1 = stream.tile([P, ncap], F32, name=f"silu_{it}", tag="silu", bufs=sm_bufs)
                nc.scalar.activation(silu1[:], ph1[:], mybir.ActivationFunctionType.Silu)
                act = persist.tile([P, ncap], F32R, name=f"act_{it}", tag=f"act_{it}")
                nc.vector.tensor_mul(act[:], silu1[:], ph3[:])
                act_sb.append(act)

            # ---- w2 stream: queued behind the w13 stream, lands while the
            # tail of stage 1 and the first h-tiles of stage 2 execute ----
            w2_grp = []
            for g in range(NI // 4):
                w2g = persist.tile([P, 4 * H], F32R, name=f"w2g_{g}", tag=f"w2g_{g}")
                nc.sync.dma_start(out=w2g[:], in_=w2_in[g].bitcast(F32R))
                w2_grp.append(w2g)
            for ii in range(NI):
                g, j = divmod(ii, 4)
                w2_sb.append(w2_grp[g][:, j * H : (j + 1) * H])

            # ---- stage 2: down-projection, ii-major so the accumulators
            # advance as each w2 group lands. For ncap<=512 a [P, ncap] psum
            # tile is a single bank, so all 8 h-tile accumulators fit in PSUM
            # at once (4 reuse stage-1's tags, 4 fresh); the dense fallback
            # needs 2 banks per tile and runs in two passes of 4. ----
            if ncap <= 512:
                GRP = NH
                tags2 = ["pnA", "pnB", "pnC", "pnD", "psA", "psB", "psC", "psD"]
            else:
                GRP = 4
                tags2 = ["psA", "psB", "psC", "psD"]
            TAIL = 4  # per-ht closing groups, staggered so out-muls overlap MMs
            for g in range(NH // GRP):
                hts = range(g * GRP, (g + 1) * GRP)
                pos = [
                    psum.tile([P, ncap], F32, name=f"po_{ht}", tag=tags2[j], bufs=1)
                    for j, ht in enumerate(hts)
                ]

                def dmm(j, ht, ii):
                    lhsT = w2_sb[ii][:, ht * P : (ht + 1) * P]
                    for ts in blks:
                        nc.tensor.matmul(
                            pos[j][:, ts], lhsT, act_sb[ii][:, ts],
                            start=(ii == 0), stop=(ii == NI - 1),
                        )

                for ii in range(NI - TAIL):
                    for j, ht in enumerate(hts):
                        dmm(j, ht, ii)
                for j, ht in enumerate(hts):
                    for ii in range(NI - TAIL, NI):
                        dmm(j, ht, ii)
                    outsb = stream.tile(
                        [P, ncap], F32, name=f"outsb_{ht}", tag="outsb", bufs=sm_bufs
                    )
                    nc.vector.tensor_mul(outsb[:], pos[j][:], comb_sb[:])
                    nc.sync.dma_start(out=outp[ht * P : (ht + 1) * P, :], in_=outsb[:])

    nc.compile()
    _NC_CACHE[ncap] = nc
    return nc


def _route(x: np.ndarray, gw: np.ndarray) -> np.ndarray:
    """Host router: softmax over expert logits, top-2, renormalize.

    Returns combine [T, E] f32 with zeros for unselected experts.
    """
    logits = x @ gw.T                                   # [T, E]
    logits = logits - logits.max(axis=1, keepdims=True)
    ex = np.exp(logits)
    rw = ex / ex.sum(axis=1, keepdims=True)
    idx = np.argsort(-rw, axis=1, kind="stable")[:, :TOPK]
    v = np.take_along_axis(rw, idx, axis=1)
    v = v / v.sum(axis=1, keepdims=True)
    combine = np.zeros((T, E), np.float32)
    np.put_along_axis(combine, idx, v.astype(np.float32), axis=1)
    return combine


def _pack_weights(wsl: np.ndarray) -> list:
    """wsl: [E, 3*I*H] -> per-expert (w13q [NI,P,2*NKH*P], w2q [NI,P,H])."""
    packs = []
    for c in range(N_CORES):
        w1 = wsl[c, : I * H].reshape(I, H)
        w3 = wsl[c, I * H : 2 * I * H].reshape(I, H)
        w2 = wsl[c, 2 * I * H :].reshape(H, I)
        w13q = np.empty((NI, P, 2 * NKH * P), np.float32)
        # lhsT blocks: w13q[it, p, kh*P+m] = w[it*P+m, kh*P+p]
        w13q[:, :, : NKH * P] = (
            w1.reshape(NI, P, NKH, P).transpose(0, 3, 2, 1).reshape(NI, P, NKH * P)
        )
        w13q[:, :, NKH * P :] = (
            w3.reshape(NI, P, NKH, P).transpose(0, 3, 2, 1).reshape(NI, P, NKH * P)
        )
        # w2 rows grouped: w2q[g, p, j*H+h] = w2[h, (4g+j)*P+p]
        w2q = np.ascontiguousarray(
            np.ascontiguousarray(w2.T)
            .reshape(NI // 4, 4, P, H)
            .transpose(0, 2, 1, 3)
        ).reshape(NI // 4, P, 4 * H)
        packs.append((w13q, w2q))
    return packs


def prepare_in_maps(index, hidden_states, gate_w, ws):
    x = np.ascontiguousarray(np.asarray(hidden_states, dtype=np.float32))
    li = int(index)
    gw = np.asarray(gate_w, dtype=np.float32)[li]       # [E, H]
    wsl = np.asarray(ws, dtype=np.float32)[li]          # [E, 3*I*H]

    combine = _route(x, gw)
    counts = (combine > 0).sum(axis=0)
    ncap = NCAP if counts.max() <= NCAP else T

    xt = np.ascontiguousarray(x.T)                      # [H, T]
    packs = _pack_weights(wsl)

    in_maps = []
    toks_list = []
    for c in range(N_CORES):
        if ncap == T:
            toks = np.arange(T)
        else:
            toks = np.nonzero(combine[:, c] > 0)[0]
        n_c = len(toks)
        xg = np.zeros((H, ncap), np.float32)
        xg[:, :n_c] = xt[:, toks]
        comb_c = np.zeros((P, ncap), np.float32)
        comb_c[:, :n_c] = combine[toks, c][None, :]
        w13q, w2q = packs[c]
        in_maps.append({"xg": xg, "comb": comb_c, "w13q": w13q, "w2q": w2q})
        toks_list.append(toks)
    return in_maps, toks_list, ncap


def run_device(in_maps, ncap, **spmd_kwargs):
    nc = build_nc(ncap)
    return run_bass_kernel_spmd(nc, in_maps, list(range(N_CORES)), **spmd_kwargs)


def assemble_output(results, toks_list) -> np.ndarray:
    outT = np.zeros((T, H), np.float32)
    for c in range(N_CORES):
        toks = toks_list[c]
        outT[toks, :] += results[c]["outp"][:, : len(toks)].T
    return outT


def kernel(index, hidden_states, experts_cache, gate_w, ws) -> np.ndarray:
    in_maps, toks_list, ncap = prepare_in_maps(index, hidden_states, gate_w, ws)
    res = run_device(in_maps, ncap)
    return assemble_output(res.results, toks_list)


# revision 23
# speedup vs baseline: 1.0238x; 1.0238x over previous
"""Mixtral MoE layer (T=1024, H=1024, I=2048, E=8, top-2) on 8 Trainium2 cores.

Strategy: token-sparse expert-parallel. The router (softmax + top-2 +
renormalize -> combine[T, E]) runs on host. Core c owns expert c's FFN and
processes only the tokens routed to expert c (on average T*K/E = 256,
padded to a fixed bucket NCAP=384; zero-padded columns contribute nothing).
Host gathers each expert's token columns of x^T (the "token all-to-all"
shard step), the device computes

    outT_c = (w2_c @ (silu(w1_c @ xg) * (w3_c @ xg))) * combine[toks_c, c]

and host scatter-adds the per-expert [H, n_c] panels back into the full
[T, H] output (the unshard step). If any expert overflows the bucket
(never for 8 experts at these sizes unless routing is degenerate), we fall
back to a dense variant: every core processes all T tokens with its
combine column, same scatter-add (toks = arange(T)).

Matmuls run as float32r (TF32-like precision, ~2.5e-4 rel err end to end,
full PE rate for moving dims >= 256). Weights are repacked on host so each
i-tile's w1/w3 lhsT blocks and w2 rows form one contiguous [128, 12KB]
DMA (~1.5 MiB per dma_start, descriptor-efficient).
"""

import os
import sys

sys.path.insert(0, "/opt/trn_rl_repo")

import numpy as np

import concourse.bacc as bacc
import concourse.tile as tile
from concourse import mybir
from concourse.bass_utils import run_bass_kernel_spmd

F32 = mybir.dt.float32
F32R = mybir.dt.float32r

T = 1024   # tokens
H = 1024   # hidden
I = 2048   # intermediate
E = 8      # experts
TOPK = 2
P = 128
NKH = H // P     # 8  h-tiles (up-proj contraction)
NI = I // P      # 16 i-tiles
NH = H // P      # 8  h-tiles (down-proj output)
N_CORES = 8
NCAP = 384       # token bucket per expert (seed-robust: mean 256, std ~14)
WCOLS = 3 * NKH * P  # packed weight row: w1 blocks | w3 blocks | w2 rows

_NC_CACHE = {}


def build_nc(ncap: int):
    if ncap in _NC_CACHE:
        return _NC_CACHE[ncap]

    # moving-operand blocks of <=512 (>=256 keeps float32r at full rate)
    nblk = (ncap + 511) // 512
    blks = []
    for b in range(nblk):
        lo = b * 512
        blks.append(slice(lo, min(lo + 512, ncap)))

    # SBUF gets tight in the dense fallback (ncap=T); shrink stream depths
    wq_bufs = 10 if ncap <= 512 else 2
    sm_bufs = 2 if ncap <= 512 else 1

    nc = bacc.Bacc(None, target_bir_lowering=False, num_devices=N_CORES)

    xg_in = nc.declare_dram_parameter("xg", [H, ncap], F32, isOutput=False)
    comb_in = nc.declare_dram_parameter("comb", [P, ncap], F32, isOutput=False)
    # per i-tile packed up-proj weights: [w1 lhsT (NKH*P) | w3 lhsT (NKH*P)]
    w13_in = nc.declare_dram_parameter("w13q", [NI, P, 2 * NKH * P], F32, isOutput=False)
    # down-proj rows in 4 coalesced groups of 4 i-tiles, streamed after the
    # w13 stream (overlaps stage 2)
    w2_in = nc.declare_dram_parameter("w2q", [NI // 4, P, 4 * H], F32, isOutput=False)
    outp = nc.declare_dram_parameter("outp", [H, ncap], F32, isOutput=True)

    with tile.TileContext(nc) as tc:
        with (
            tc.tile_pool(name="persist", bufs=1) as persist,
            tc.tile_pool(name="stream", bufs=2) as stream,
            tc.tile_pool(name="psum", bufs=1, space="PSUM") as psum,
        ):
            # first i-tile's weights ahead of the xg block so PE starts early;
            # split so the w1 blocks (first consumers) land first
            WB = NKH * P
            wq0 = stream.tile([P, 2 * WB], F32R, name="wq_0", tag="wq", bufs=wq_bufs)
            nc.sync.dma_start(out=wq0[:, 0:WB], in_=w13_in[0][:, 0:WB].bitcast(F32R))

            xg_sb = []

            def load_xg(kh):
                t_ = persist.tile([P, ncap], F32R, name=f"xg_{kh}", tag=f"xg_{kh}")
                nc.sync.dma_start(
                    out=t_[:], in_=xg_in[kh * P : (kh + 1) * P, :].bitcast(F32R)
                )
                xg_sb.append(t_)

            for kh in range(3):
                load_xg(kh)
            nc.sync.dma_start(
                out=wq0[:, WB : 2 * WB], in_=w13_in[0][:, WB : 2 * WB].bitcast(F32R)
            )
            load_xg(3)
            wq1 = stream.tile([P, 2 * WB], F32R, name="wq_1", tag="wq", bufs=wq_bufs)
            nc.sync.dma_start(out=wq1[:], in_=w13_in[1].bitcast(F32R))
            for kh in range(4, NKH):
                load_xg(kh)
            comb_sb = persist.tile([P, ncap], F32, name="comb_sb", tag="comb_sb")
            nc.sync.dma_start(out=comb_sb[:], in_=comb_in[:, :])

            w2_sb = []
            act_sb = []

            # ---- stage 1: up-projections + SwiGLU, per i-tile ----
            for it in range(NI):
                if it == 0:
                    wq = wq0
                elif it == 1:
                    wq = wq1
                else:
                    wq = stream.tile([P, 2 * WB], F32R, name=f"wq_{it}", tag="wq", bufs=wq_bufs)
                    nc.sync.dma_start(out=wq[:], in_=w13_in[it].bitcast(F32R))

                # 4 psum tags cycle between ph1/ph3 (alternating => double
                # buffered) and are reused by stage 2's 4 accumulators
                ph1 = psum.tile([P, ncap], F32, name=f"ph1_{it}",
                                tag=f"ps{'AB'[it % 2]}", bufs=1)
                ph3 = psum.tile([P, ncap], F32, name=f"ph3_{it}",
                                tag=f"ps{'CD'[it % 2]}", bufs=1)
                # alternate the two accumulator banks between consecutive PE
                # writes; i-tile 0 runs ph-major instead so its w1-block MMs
                # proceed while the w3 half of wq0 is still streaming in
                if it == 0:
                    order = [(ph, woff, kh) for ph, woff in ((ph1, 0), (ph3, NKH * P))
                             for kh in range(NKH)]
                else:
                    order = [(ph, woff, kh) for kh in range(NKH)
                             for ph, woff in ((ph1, 0), (ph3, NKH * P))]
                for ph, woff, kh in order:
                    lhsT = wq[:, woff + kh * P : woff + (kh + 1) * P]
                    for ts in blks:
                        nc.tensor.matmul(
                            ph[:, ts], lhsT, xg_sb[kh][:, ts],
                            start=(kh == 0), stop=(kh == NKH - 1),
                        )

                silu1 = stream.tile([P, ncap], F32, name=f"silu_{it}", tag="silu", bufs=sm_bufs)
                nc.scalar.activation(silu1[:], ph1[:], mybir.ActivationFunctionType.Silu)
                act = persist.tile([P, ncap], F32R, name=f"act_{it}", tag=f"act_{it}")
                nc.vector.tensor_mul(act[:], silu1[:], ph3[:])
                act_sb.append(act)

            # ---- w2 stream: queued behind the w13 stream, lands while the
            # tail of stage 1 and the first h-tiles of stage 2 execute ----
            w2_grp = []
            for g in range(NI // 4):
                w2g = persist.tile([P, 4 * H], F32R, name=f"w2g_{g}", tag=f"w2g_{g}")
                nc.sync.dma_start(out=w2g[:], in_=w2_in[g].bitcast(F32R))
                w2_grp.append(w2g)
            for ii in range(NI):
                g, j = divmod(ii, 4)
                w2_sb.append(w2_grp[g][:, j * H : (j + 1) * H])

            # ---- stage 2: down-projection, ii-major so the accumulators
            # advance as each w2 group lands. For ncap<=512 a [P, ncap] psum
            # tile is a single bank, so all 8 h-tile accumulators fit in PSUM
            # at once (4 reuse stage-1's tags, 4 fresh); the dense fallback
            # needs 2 banks per tile and runs in two passes of 4. ----
            if ncap <= 512:
                GRP = NH
                tags2 = ["pnA", "pnB", "pnC", "pnD", "psA", "psB", "psC", "psD"]
            else:
                GRP = 4
                tags2 = ["psA", "psB", "psC", "psD"]
            TAIL = 4  # per-ht closing groups, staggered so out-muls overlap MMs
            for g in range(NH // GRP):
                hts = range(g * GRP, (g + 1) * GRP)
                pos = [
                    psum.tile([P, ncap], F32, name=f"po_{ht}", tag=tags2[j], bufs=1)
                    for j, ht in enumerate(hts)
                ]

                def dmm(j, ht, ii):
                    lhsT = w2_sb[ii][:, ht * P : (ht + 1) * P]
                    for ts in blks:
                        nc.tensor.matmul(
                            pos[j][:, ts], lhsT, act_sb[ii][:, ts],
                            start=(ii == 0), stop=(ii == NI - 1),
                        )

                for ii in range(NI - TAIL):
                    for j, ht in enumerate(hts):
                        dmm(j, ht, ii)
                for j, ht in enumerate(hts):
                    for ii in range(NI - TAIL, NI):
                        dmm(j, ht, ii)
                    outsb = stream.tile(
                        [P, ncap], F32, name=f"outsb_{ht}", tag="outsb", bufs=sm_bufs
                    )
                    nc.vector.tensor_mul(outsb[:], pos[j][:], comb_sb[:])
                    nc.sync.dma_start(out=outp[ht * P : (ht + 1) * P, :], in_=outsb[:])

    nc.compile()
    _NC_CACHE[ncap] = nc
    return nc


def _route(x: np.ndarray, gw: np.ndarray) -> np.ndarray:
    """Host router: softmax over expert logits, top-2, renormalize.

    Returns combine [T, E] f32 with zeros for unselected experts.
    """
    logits = x @ gw.T                                   # [T, E]
    logits = logits - logits.max(axis=1, keepdims=True)
    ex = np.exp(logits)
    rw = ex / ex.sum(axis=1, keepdims=True)
    idx = np.argsort(-rw, axis=1, kind="stable")[:, :TOPK]
    v = np.take_along_axis(rw, idx, axis=1)
    v = v / v.sum(axis=1, keepdims=True)
    combine = np.zeros((T, E), np.float32)
    np.put_along_axis(combine, idx, v.astype(np.float32), axis=1)
    return combine


def _pack_weights(wsl: np.ndarray) -> list:
    """wsl: [E, 3*I*H] -> per-expert (w13q [NI,P,2*NKH*P], w2q [NI,P,H])."""
    packs = []
    for c in range(N_CORES):
        w1 = wsl[c, : I * H].reshape(I, H)
        w3 = wsl[c, I * H : 2 * I * H].reshape(I, H)
        w2 = wsl[c, 2 * I * H :].reshape(H, I)
        w13q = np.empty((NI, P, 2 * NKH * P), np.float32)
        # lhsT blocks: w13q[it, p, kh*P+m] = w[it*P+m, kh*P+p]
        w13q[:, :, : NKH * P] = (
            w1.reshape(NI, P, NKH, P).transpose(0, 3, 2, 1).reshape(NI, P, NKH * P)
        )
        w13q[:, :, NKH * P :] = (
            w3.reshape(NI, P, NKH, P).transpose(0, 3, 2, 1).reshape(NI, P, NKH * P)
        )
        # w2 rows grouped: w2q[g, p, j*H+h] = w2[h, (4g+j)*P+p]
        w2q = np.ascontiguousarray(
            np.ascontiguousarray(w2.T)
            .reshape(NI // 4, 4, P, H)
            .transpose(0, 2, 1, 3)
        ).reshape(NI // 4, P, 4 * H)
        packs.append((w13q, w2q))
    return packs


def prepare_in_maps(index, hidden_states, gate_w, ws):
    x = np.ascontiguousarray(np.asarray(hidden_states, dtype=np.float32))
    li = int(index)
    gw = np.asarray(gate_w, dtype=np.float32)[li]       # [E, H]
    wsl = np.asarray(ws, dtype=np.float32)[li]          # [E, 3*I*H]

    combine = _route(x, gw)
    counts = (combine > 0).sum(axis=0)
    ncap = NCAP if counts.max() <= NCAP else T

    xt = np.ascontiguousarray(x.T)                      # [H, T]
    packs = _pack_weights(wsl)

    in_maps = []
    toks_list = []
    for c in range(N_CORES):
        if ncap == T:
            toks = np.arange(T)
        else:
            toks = np.nonzero(combine[:, c] > 0)[0]
        n_c = len(toks)
        xg = np.zeros((H, ncap), np.float32)
        xg[:, :n_c] = xt[:, toks]
        comb_c = np.zeros((P, ncap), np.float32)
        comb_c[:, :n_c] = combine[toks, c][None, :]
        w13q, w2q = packs[c]
        in_maps.append({"xg": xg, "comb": comb_c, "w13q": w13q, "w2q": w2q})
        toks_list.append(toks)
    return in_maps, toks_list, ncap


def run_device(in_maps, ncap, **spmd_kwargs):
    nc = build_nc(ncap)
    return run_bass_kernel_spmd(nc, in_maps, list(range(N_CORES)), **spmd_kwargs)


def assemble_output(results, toks_list) -> np.ndarray:
    outT = np.zeros((T, H), np.float32)
    for c in range(N_CORES):
        toks = toks_list[c]
        outT[toks, :] += results[c]["outp"][:, : len(toks)].T
    return outT


def kernel(index, hidden_states, experts_cache, gate_w, ws) -> np.ndarray:
    in_maps, toks_list, ncap = prepare_in_maps(index, hidden_states, gate_w, ws)
    res = run_device(in_maps, ncap)
    return assemble_output(res.results, toks_list)


# revision 24
# speedup vs baseline: 1.1564x; 1.1295x over previous
"""Mixtral MoE layer (T=1024, H=1024, I=2048, E=8, top-2) on 8 Trainium2 cores.

Strategy: token-sparse expert-parallel. The router (softmax + top-2 +
renormalize -> combine[T, E]) runs on host. Core c owns expert c's FFN and
processes only the tokens routed to expert c (on average T*K/E = 256,
padded to a fixed bucket NCAP=384; zero-padded columns contribute nothing).
Host gathers each expert's token columns of x^T (the "token all-to-all"
shard step), the device computes

    outT_c = (w2_c @ (silu(w1_c @ xg) * (w3_c @ xg))) * combine[toks_c, c]

and host scatter-adds the per-expert [H, n_c] panels back into the full
[T, H] output (the unshard step). If any expert overflows the bucket
(never for 8 experts at these sizes unless routing is degenerate), we fall
back to a dense variant: every core processes all T tokens with its
combine column, same scatter-add (toks = arange(T)).

Matmuls run as float32r (TF32-like precision, ~2.5e-4 rel err end to end,
full PE rate for moving dims >= 256). Weights are repacked on host so each
i-tile's w1/w3 lhsT blocks and w2 rows form one contiguous [128, 12KB]
DMA (~1.5 MiB per dma_start, descriptor-efficient).
"""

import os
import sys

sys.path.insert(0, "/opt/trn_rl_repo")

import numpy as np

import concourse.bacc as bacc
import concourse.tile as tile
from concourse import mybir
from concourse.bass_utils import run_bass_kernel_spmd

F32 = mybir.dt.float32
F32R = mybir.dt.float32r

T = 1024   # tokens
H = 1024   # hidden
I = 2048   # intermediate
E = 8      # experts
TOPK = 2
P = 128
NKH = H // P     # 8  h-tiles (up-proj contraction)
NI = I // P      # 16 i-tiles
NH = H // P      # 8  h-tiles (down-proj output)
N_CORES = 8
NCAP = 384       # token bucket per expert (seed-robust: mean 256, std ~14)
WCOLS = 3 * NKH * P  # packed weight row: w1 blocks | w3 blocks | w2 rows

_NC_CACHE = {}


def build_nc(ncap: int):
    if ncap in _NC_CACHE:
        return _NC_CACHE[ncap]

    # moving-operand blocks of <=512 (>=256 keeps float32r at full rate)
    nblk = (ncap + 511) // 512
    blks = []
    for b in range(nblk):
        lo = b * 512
        blks.append(slice(lo, min(lo + 512, ncap)))

    # SBUF gets tight in the dense fallback (ncap=T); shrink stream depths
    wq_bufs = 8 if ncap <= 512 else 2
    sm_bufs = 2 if ncap <= 512 else 1

    nc = bacc.Bacc(None, target_bir_lowering=False, num_devices=N_CORES)

    xg_in = nc.declare_dram_parameter("xg", [H, ncap], F32, isOutput=False)
    comb_in = nc.declare_dram_parameter("comb", [P, ncap], F32, isOutput=False)
    # per i-tile packed up-proj weights: [w1 lhsT (NKH*P) | w3 lhsT (NKH*P)]
    w13_in = nc.declare_dram_parameter("w13q", [NI, P, 2 * NKH * P], F32, isOutput=False)
    # down-proj rows in 4 coalesced groups of 4 i-tiles, streamed after the
    # w13 stream (overlaps stage 2)
    w2_in = nc.declare_dram_parameter("w2q", [NI // 4, P, 4 * H], F32, isOutput=False)
    outp = nc.declare_dram_parameter("outp", [H, ncap], F32, isOutput=True)

    with tile.TileContext(nc) as tc:
        with (
            tc.tile_pool(name="persist", bufs=1) as persist,
            tc.tile_pool(name="stream", bufs=2) as stream,
            tc.tile_pool(name="psum", bufs=1, space="PSUM") as psum,
        ):
            # first i-tile's weights ahead of the xg block so PE starts early;
            # split so the w1 blocks (first consumers) land first
            WB = NKH * P
            wq0 = stream.tile([P, 2 * WB], F32R, name="wq_0", tag="wq", bufs=wq_bufs)
            nc.sync.dma_start(out=wq0[:, 0:WB], in_=w13_in[0][:, 0:WB].bitcast(F32R))

            xg_sb = []

            def load_xg(kh):
                t_ = persist.tile([P, ncap], F32R, name=f"xg_{kh}", tag=f"xg_{kh}")
                nc.sync.dma_start(
                    out=t_[:], in_=xg_in[kh * P : (kh + 1) * P, :].bitcast(F32R)
                )
                xg_sb.append(t_)

            for kh in range(3):
                load_xg(kh)
            nc.sync.dma_start(
                out=wq0[:, WB : 2 * WB], in_=w13_in[0][:, WB : 2 * WB].bitcast(F32R)
            )
            load_xg(3)
            wq1 = stream.tile([P, 2 * WB], F32R, name="wq_1", tag="wq", bufs=wq_bufs)
            nc.sync.dma_start(out=wq1[:], in_=w13_in[1].bitcast(F32R))
            for kh in range(4, NKH):
                load_xg(kh)
            comb_sb = persist.tile([P, ncap], F32, name="comb_sb", tag="comb_sb")
            nc.sync.dma_start(out=comb_sb[:], in_=comb_in[:, :])

            w2_sb = []
            act_sb = []

            # ---- stage 1: up-projections + SwiGLU, per i-tile ----
            for it in range(NI):
                if it == 0:
                    wq = wq0
                elif it == 1:
                    wq = wq1
                else:
                    wq = stream.tile([P, 2 * WB], F32R, name=f"wq_{it}", tag="wq", bufs=wq_bufs)
                    nc.sync.dma_start(out=wq[:], in_=w13_in[it].bitcast(F32R))

                # 4 psum tags cycle between ph1/ph3 (alternating => double
                # buffered) and are reused by stage 2's 4 accumulators
                ph1 = psum.tile([P, ncap], F32, name=f"ph1_{it}",
                                tag=f"ps{'AB'[it % 2]}", bufs=1)
                ph3 = psum.tile([P, ncap], F32, name=f"ph3_{it}",
                                tag=f"ps{'CD'[it % 2]}", bufs=1)
                # alternate the two accumulator banks so consecutive PE
                # writes never target the same PSUM bank
                for kh in range(NKH):
                    for ph, woff in ((ph1, 0), (ph3, NKH * P)):
                        lhsT = wq[:, woff + kh * P : woff + (kh + 1) * P]
                        for ts in blks:
                            nc.tensor.matmul(
                                ph[:, ts], lhsT, xg_sb[kh][:, ts],
                                start=(kh == 0), stop=(kh == NKH - 1),
                            )

                silu1 = stream.tile([P, ncap], F32, name=f"silu_{it}", tag="silu", bufs=sm_bufs)
                nc.scalar.activation(silu1[:], ph1[:], mybir.ActivationFunctionType.Silu)
                act = persist.tile([P, ncap], F32R, name=f"act_{it}", tag=f"act_{it}")
                nc.vector.tensor_mul(act[:], silu1[:], ph3[:])
                act_sb.append(act)

            # ---- w2 stream: queued behind the w13 stream, lands while the
            # tail of stage 1 and the first h-tiles of stage 2 execute ----
            w2_grp = []
            for g in range(NI // 4):
                w2g = persist.tile([P, 4 * H], F32R, name=f"w2g_{g}", tag=f"w2g_{g}")
                nc.sync.dma_start(out=w2g[:], in_=w2_in[g].bitcast(F32R))
                w2_grp.append(w2g)
            for ii in range(NI):
                g, j = divmod(ii, 4)
                w2_sb.append(w2_grp[g][:, j * H : (j + 1) * H])

            # ---- stage 2: down-projection, ii-major so the accumulators
            # advance as each w2 group lands. For ncap<=512 a [P, ncap] psum
            # tile is a single bank, so all 8 h-tile accumulators fit in PSUM
            # at once (4 reuse stage-1's tags, 4 fresh); the dense fallback
            # needs 2 banks per tile and runs in two passes of 4. ----
            if ncap <= 512:
                GRP = NH
                tags2 = ["pnA", "pnB", "pnC", "pnD", "psA", "psB", "psC", "psD"]
            else:
                GRP = 4
                tags2 = ["psA", "psB", "psC", "psD"]
            TAIL = 4  # per-ht closing groups, staggered so out-muls overlap MMs
            for g in range(NH // GRP):
                hts = range(g * GRP, (g + 1) * GRP)
                pos = [
                    psum.tile([P, ncap], F32, name=f"po_{ht}", tag=tags2[j], bufs=1)
                    for j, ht in enumerate(hts)
                ]

                def dmm(j, ht, ii):
                    lhsT = w2_sb[ii][:, ht * P : (ht + 1) * P]
                    for ts in blks:
                        nc.tensor.matmul(
                            pos[j][:, ts], lhsT, act_sb[ii][:, ts],
                            start=(ii == 0), stop=(ii == NI - 1),
                        )

                for ii in range(NI - TAIL):
                    for j, ht in enumerate(hts):
                        dmm(j, ht, ii)
                for j, ht in enumerate(hts):
                    for ii in range(NI - TAIL, NI):
                        dmm(j, ht, ii)
                    outsb = stream.tile(
                        [P, ncap], F32, name=f"outsb_{ht}", tag="outsb", bufs=sm_bufs
                    )
                    nc.vector.tensor_mul(outsb[:], pos[j][:], comb_sb[:])
                    nc.sync.dma_start(out=outp[ht * P : (ht + 1) * P, :], in_=outsb[:])

    nc.compile()
    _NC_CACHE[ncap] = nc
    return nc


def _route(x: np.ndarray, gw: np.ndarray) -> np.ndarray:
    """Host router: softmax over expert logits, top-2, renormalize.

    Returns combine [T, E] f32 with zeros for unselected experts.
    """
    logits = x @ gw.T                                   # [T, E]
    logits = logits - logits.max(axis=1, keepdims=True)
    ex = np.exp(logits)
    rw = ex / ex.sum(axis=1, keepdims=True)
    idx = np.argsort(-rw, axis=1, kind="stable")[:, :TOPK]
    v = np.take_along_axis(rw, idx, axis=1)
    v = v / v.sum(axis=1, keepdims=True)
    combine = np.zeros((T, E), np.float32)
    np.put_along_axis(combine, idx, v.astype(np.float32), axis=1)
    return combine


def _pack_weights(wsl: np.ndarray) -> list:
    """wsl: [E, 3*I*H] -> per-expert (w13q [NI,P,2*NKH*P], w2q [NI,P,H])."""
    packs = []
    for c in range(N_CORES):
        w1 = wsl[c, : I * H].reshape(I, H)
        w3 = wsl[c, I * H : 2 * I * H].reshape(I, H)
        w2 = wsl[c, 2 * I * H :].reshape(H, I)
        w13q = np.empty((NI, P, 2 * NKH * P), np.float32)
        # lhsT blocks: w13q[it, p, kh*P+m] = w[it*P+m, kh*P+p]
        w13q[:, :, : NKH * P] = (
            w1.reshape(NI, P, NKH, P).transpose(0, 3, 2, 1).reshape(NI, P, NKH * P)
        )
        w13q[:, :, NKH * P :] = (
            w3.reshape(NI, P, NKH, P).transpose(0, 3, 2, 1).reshape(NI, P, NKH * P)
        )
        # w2 rows grouped: w2q[g, p, j*H+h] = w2[h, (4g+j)*P+p]
        w2q = np.ascontiguousarray(
            np.ascontiguousarray(w2.T)
            .reshape(NI // 4, 4, P, H)
            .transpose(0, 2, 1, 3)
        ).reshape(NI // 4, P, 4 * H)
        packs.append((w13q, w2q))
    return packs


def prepare_in_maps(index, hidden_states, gate_w, ws):
    x = np.ascontiguousarray(np.asarray(hidden_states, dtype=np.float32))
    li = int(index)
    gw = np.asarray(gate_w, dtype=np.float32)[li]       # [E, H]
    wsl = np.asarray(ws, dtype=np.float32)[li]          # [E, 3*I*H]

    combine = _route(x, gw)
    counts = (combine > 0).sum(axis=0)
    ncap = NCAP if counts.max() <= NCAP else T

    xt = np.ascontiguousarray(x.T)                      # [H, T]
    packs = _pack_weights(wsl)

    in_maps = []
    toks_list = []
    for c in range(N_CORES):
        if ncap == T:
            toks = np.arange(T)
        else:
            toks = np.nonzero(combine[:, c] > 0)[0]
        n_c = len(toks)
        xg = np.zeros((H, ncap), np.float32)
        xg[:, :n_c] = xt[:, toks]
        comb_c = np.zeros((P, ncap), np.float32)
        comb_c[:, :n_c] = combine[toks, c][None, :]
        w13q, w2q = packs[c]
        in_maps.append({"xg": xg, "comb": comb_c, "w13q": w13q, "w2q": w2q})
        toks_list.append(toks)
    return in_maps, toks_list, ncap


def run_device(in_maps, ncap, **spmd_kwargs):
    nc = build_nc(ncap)
    return run_bass_kernel_spmd(nc, in_maps, list(range(N_CORES)), **spmd_kwargs)


def assemble_output(results, toks_list) -> np.ndarray:
    outT = np.zeros((T, H), np.float32)
    for c in range(N_CORES):
        toks = toks_list[c]
        outT[toks, :] += results[c]["outp"][:, : len(toks)].T
    return outT


def kernel(index, hidden_states, experts_cache, gate_w, ws) -> np.ndarray:
    in_maps, toks_list, ncap = prepare_in_maps(index, hidden_states, gate_w, ws)
    res = run_device(in_maps, ncap)
    return assemble_output(res.results, toks_list)
